# revision 29
# baseline (speedup 1.0000x reference)
"""ConvSelfAttention Trainium2 kernel.

Reference computation (per batch b, with x flattened to [C=128, N=4096]):
    q = wq @ x + bq        [64, N]   (scaled by 1/sqrt(128), folded into wq/bq)
    k = wk @ x + bk        [64, N]
    v = wv @ x + bv        [64, N]
    s[i,j] = sum_o q[o,i] k[o,j]
    p = softmax_j(s)
    out[o,i] = sum_j v[o,j] p[i,j]
    y = gamma * (wo @ out + bo) + x

Mapping (one batch per NeuronCore, 8 cores):
  - scores are built TRANSPOSED: sT[j,i] = sum_o k[o,j] q[o,i], j-tile (128) on
    partitions, i-block (512) on free dim; exp is SPLIT by score-group between
    ScalarE (Act.Exp, fp8e4m3 out) and VectorE (Schraudolph exp2 bit trick:
    int8(round(A*s + B)) bitcast to fp8e4m3) so the two engines share the
    16.8M-element softmax wall.  Emission is software-pipelined with a +2
    group skew (QK(s) | exp(s-1) | PV(s-2)) so the PE never head-of-line
    waits on exp and stays HAM-warm.
  - QK has K=64 (head dim), so q/k are kept DUPLICATED in both partition
    halves (duplication is free: the projection weight matrix is duplicated on
    the host) and consecutive j-tiles run CONCURRENTLY in the PE array via
    row tile_position (0,0)/(64,0).
  - PV uses the exp output pT (fp8) as the STATIONARY operand (M=128
    i-columns, FWL loads fp8 weights 4/cycle) streaming the ones-augmented
    fp8 V^T (N=65):
    psum[i, 0:64] accumulates attention output (transposed), psum[i, 64] the
    softmax denominator -- per-partition, so normalization is a plain
    tensor_scalar multiply and the reciprocal is a [128, 4] op.
  - the normalized [i, o] tile is PE-transposed back to [o, i] for the output
    projection; v's bias folds into the constant gamma*(wo@bv+bo) because
    sum_j p = 1; gamma folds into wo/bo on the host.

All matmuls run in bf16 (fp32 PSUM accumulation); exp runs on ScalarE in fp32.
"""

import sys

import numpy as np

try:
    import concourse  # noqa: F401
except ImportError:  # pragma: no cover
    sys.path.insert(0, "/opt/trn_rl_repo")

import ml_dtypes

B, C, CO, N = 8, 128, 64, 4096
W = H = 64
NCORES = 8
IBLK = 512          # query columns per i-block
NSUB = IBLK // 128  # 4 i-subtiles per i-block (PV stationary operand width)
NJT = N // 128      # 32 j-tiles of 128 keys
NIB = N // IBLK     # 8 i-blocks
JGRP = 2            # j-tiles per exp group (2 PSUM banks; 3 bufs in flight)
NGRP = NJT // JGRP  # 16 score groups per i-block
# fp8e4m3 exp2 bit trick: bits = round(8*log2e*s + (7*8 - C)), C tuned so the
# piecewise-linear 2^frac interpolation error is centered
EXP_A8 = 8.0 / float(np.log(2.0))
EXP_B8 = 7.0 * 8.0 - 0.35
# score columns per group handled by ScalarE (true exp); the rest go to
# VectorE via the bit trick.  576/448 balances the engines including VectorE's
# epilogue/setup duties.
XSPL = 576

_CACHE = {}


def _split_multiwaits(nc):
    """Workaround for the pinned walrus: it accepts at most ONE semaphore wait
    per instruction (setupSyncWait: "Too many sync wait commands").  Hoist all
    but the last wait of any instruction onto single-wait NoOps inserted just
    before it in the same engine's stream — semantically identical (the engine
    blocks on each wait in turn before issuing the instruction)."""
    from concourse import mybir

    nsplit = 0
    for fn in nc.m.functions:
        for bb in fn.blocks:
            out = []
            for inst in bb.instructions:
                si = inst.sync_info
                if si is not None and si.on_wait is not None and len(si.on_wait) > 1:
                    waits = list(si.on_wait)
                    for i, w in enumerate(waits[:-1]):
                        out.append(mybir.InstNoOp(
                            name=f"{inst.name}-sw{i}",
                            engine=inst.engine,
                            sync_info=mybir.SyncInfo(on_wait=[w], on_update=[]),
                            bass_nofuse=True,
                        ))
                        nsplit += 1
                    si.on_wait = [waits[-1]]
                    inst.sync_info = si
                out.append(inst)
            bb.instructions = out
    return nsplit


def build_nc(debug=False, nib=NIB, use_bacc=False, split=True):
    from concourse import bacc, mybir
    import concourse.bass as bass
    import concourse.tile as tile
    from concourse.masks import make_identity

    f32 = mybir.dt.float32
    bf16 = mybir.dt.bfloat16
    f8 = mybir.dt.float8e4
    i8 = mybir.dt.int8
    Alu = mybir.AluOpType
    Act = mybir.ActivationFunctionType

    if use_bacc:
        nc = bacc.Bacc(None, target_bir_lowering=False, debug=debug)
    else:
        nc = bass.Bass()

    x_d = nc.dram_tensor("x", [C, N], f32, kind="ExternalInput")
    xb_d = nc.dram_tensor("xb", [C, N], bf16, kind="ExternalInput")  # host cast
    # packed bf16 weights: [wqT(128) | wkT(128) | wvT(64) | woT(128, rows 0:64)]
    wpack_d = nc.dram_tensor("wpack", [C, 448], bf16, kind="ExternalInput")
    # packed f32 scalars: [bq | bk | gbo]
    bpack_d = nc.dram_tensor("bpack", [C, 3], f32, kind="ExternalInput")
    y_d = nc.dram_tensor("y", [C, N], f32, kind="ExternalOutput")

    with tile.TileContext(nc) as tc:
        with (
            tc.tile_pool(name="consts", bufs=1) as consts,
            tc.tile_pool(name="big", bufs=1) as big,
            tc.tile_pool(name="pt", bufs=6) as pt_pool,
            tc.tile_pool(name="epi", bufs=2) as epi,
        ):
            # ---- x chunk 0 DMA first (critical path), then packed weights ----
            x_sb = big.tile([C, N], f32)
            x_bf = big.tile([C, N], bf16)
            nc.sync.dma_start(x_bf[:, 0:512], xb_d[:, 0:512])

            wpack = consts.tile([C, 448], bf16)
            nc.gpsimd.dma_start(wpack, wpack_d[:, :])
            bpack = consts.tile([C, 3], f32)
            nc.gpsimd.dma_start(bpack, bpack_d[:, :])
            wqT = wpack[:, 0:128]
            wkT = wpack[:, 128:256]
            wvT = wpack[:, 256:320]
            woT = wpack[0:CO, 320:448]
            bq_s = bpack[:, 0:1]
            bk_s = bpack[:, 1:2]
            gbo = bpack[:, 2:3]
            ident = consts.tile([C, C], bf16)
            make_identity(nc, ident)

            # warm the exp table set (~2.7us ACT_TABLE_LOAD) during the ramp;
            # memset source so no DMA dependency
            warm = consts.tile([C, 1], f32)
            nc.vector.memset(warm, 0.0)
            nc.scalar.activation(warm, warm, Act.Exp)

            # HAM pre-warm: ~3.4us of dummy matmuls while the weight/x DMAs
            # are in flight trips the PE clock gate to 8/8 (2.4 GHz) before
            # the real projections start (cold MMs run at 1.2 GHz)
            warm_x = consts.tile([C, 512], bf16)
            nc.vector.memset(warm_x, 0.0)

            # ---- x load + cast + projections, pipelined in 512-col chunks ----
            q_sb = big.tile([C, N], bf16)
            k_sb = big.tile([C, N], bf16)
            vT = big.tile([C, NJT * (CO + 1)], f8)  # 32 x [128, 65] tiles
            vT3 = vT.rearrange("p (t e) -> p t e", e=CO + 1)
            nc.vector.memset(vT3[:, :, CO:CO + 1], 1.0)

            with tc.tile_pool(name="setup_ps", bufs=4, space="PSUM") as setup_ps:
                ps_warm = setup_ps.tile([C, 512], f32, tag="warm", bufs=1)
                for _ in range(8):
                    nc.tensor.matmul(ps_warm, lhsT=warm_x[:, 0:128],
                                     rhs=warm_x, start=True, stop=True)
                for t in range(N // 512):
                    sl = slice(t * 512, (t + 1) * 512)
                    if t > 0:  # chunk 0 DMA already issued above
                        nc.sync.dma_start(x_bf[:, sl], xb_d[:, sl])
                    # x f32 (residual add) loads independently, off the
                    # projection critical path (gpsimd queue keeps the sync
                    # queue free for the xb chunks the projections wait on)
                    nc.gpsimd.dma_start(x_sb[:, sl], x_d[:, sl])
                    ps_q = setup_ps.tile([C, 512], f32, tag="proj")
                    nc.tensor.matmul(ps_q, lhsT=wqT, rhs=x_bf[:, sl],
                                     start=True, stop=True)
                    nc.vector.tensor_scalar_add(q_sb[:, sl], ps_q, bq_s)
                    ps_k = setup_ps.tile([C, 512], f32, tag="proj")
                    nc.tensor.matmul(ps_k, lhsT=wkT, rhs=x_bf[:, sl],
                                     start=True, stop=True)
                    # k bias on ScalarE (idle until the first exp) so setup
                    # is not paced by DVE alone
                    nc.scalar.activation(k_sb[:, sl], ps_k, Act.Identity,
                                         bias=bk_s)
                    ps_v = setup_ps.tile([C, 256], f32, tag="vt", bufs=2)
                    for tt in range(4):
                        nt = t * 4 + tt
                        nc.tensor.matmul(
                            ps_v[:, tt * CO:(tt + 1) * CO],
                            lhsT=x_bf[:, nt * 128:(nt + 1) * 128],
                            rhs=wvT,
                            start=True, stop=True,
                        )
                    nc.vector.tensor_copy(
                        vT3[:, t * 4:(t + 1) * 4, 0:CO],
                        ps_v.rearrange("p (t e) -> p t e", e=CO),
                    )

            # ---- main loop: software-pipelined QK(s) | exp(s-1) | PV(s-2) ---
            # The +2 emission skew keeps the PE stream free of head-of-line
            # waits (PV(g) only issues after exp(g) had a full step to run),
            # which keeps the PE HAM-warm at 2.4 GHz.
            with (
                tc.tile_pool(name="qk_ps", bufs=3, space="PSUM") as qk_ps_pool,
                tc.tile_pool(name="pv_ps", bufs=1, space="PSUM") as pv_ps_pool,
                tc.tile_pool(name="oc_ps", bufs=1, space="PSUM") as oc_ps_pool,
            ):
                groups = [(ib, g) for ib in range(nib) for g in range(NGRP)]
                nsteps = len(groups)
                qk_tiles, pT_tiles, pv_tiles, onT_tiles = {}, {}, {}, {}

                def emit_epilogue_a(ib):
                    # normalize only: these are the ops that hold the PV
                    # accumulator bank, so they go to the front of the queue
                    ps_pv = pv_tiles.pop(ib)
                    rden = epi.tile([C, NSUB], f32, tag="rden")
                    pv3 = ps_pv.rearrange("p (s e) -> p s e", e=CO + 1)
                    nc.vector.reciprocal(rden, pv3[:, :, CO])
                    onT = epi.tile([C, NSUB * CO], bf16, tag="onT")
                    onT_tiles[ib] = onT
                    for s in range(NSUB):
                        nc.vector.tensor_scalar_mul(
                            onT[:, s * CO:(s + 1) * CO],
                            pv3[:, s, 0:CO],
                            rden[:, s:s + 1],
                        )

                def emit_epilogue_b(ib):
                    # transpose, output projection, residual, store -- deferred
                    # two steps so the next i-block's exp groups queue first
                    isl = slice(ib * IBLK, (ib + 1) * IBLK)
                    onT = onT_tiles.pop(ib)
                    out_sb = epi.tile([CO, IBLK], bf16, tag="out")
                    for s in range(NSUB):
                        # transpose via a REGULAR matmul against identity
                        # (onT_s.T @ I): ~85ns vs ~275ns for transpose-mode
                        ps_t = oc_ps_pool.tile([CO, 128], f32, tag="oc")
                        nc.tensor.matmul(ps_t, lhsT=onT[:, s * CO:(s + 1) * CO],
                                         rhs=ident, start=True, stop=True)
                        nc.vector.tensor_copy(out_sb[:, s * 128:(s + 1) * 128],
                                              ps_t)
                    ps_oc = oc_ps_pool.tile([C, IBLK], f32, tag="oc")
                    nc.tensor.matmul(ps_oc, lhsT=woT, rhs=out_sb,
                                     start=True, stop=True)
                    y2 = epi.tile([C, IBLK], f32, tag="y2")
                    nc.vector.scalar_tensor_tensor(
                        out=y2, in0=ps_oc, scalar=gbo, in1=x_sb[:, isl],
                        op0=Alu.add, op1=Alu.add,
                    )
                    nc.sync.dma_start(y_d[:, isl], y2)

                for s in range(nsteps + 2):
                    # deferred epilogue phase B (two steps after its phase A)
                    if s - 4 >= 0 and groups[s - 4][1] == NGRP - 1:
                        emit_epilogue_b(groups[s - 4][0])
                    if s < nsteps:  # QK for group s
                        ib, g = groups[s]
                        isl = slice(ib * IBLK, (ib + 1) * IBLK)
                        ps_qk = qk_ps_pool.tile([128, JGRP * 512], f32)
                        qk_tiles[s] = ps_qk
                        for idx in range(JGRP):
                            jt = g * JGRP + idx
                            half = jt % 2  # alternate row halves -> PE pairs
                            hsl = slice(half * CO, half * CO + CO)
                            nc.tensor.matmul(
                                ps_qk[:, idx * 512:(idx + 1) * 512],
                                lhsT=k_sb[hsl, jt * 128:(jt + 1) * 128],
                                rhs=q_sb[hsl, isl],
                                start=True, stop=True,
                            )
                    if 0 <= s - 1 < nsteps:  # exp for group s-1
                        ib, g = groups[s - 1]
                        ps_qk = qk_tiles.pop(s - 1)
                        pT = pt_pool.tile([128, JGRP * 512], f8)
                        pT_tiles[s - 1] = pT
                        # column-split across BOTH engines so each group's
                        # exp completes in ~0.7us (vs 1.2us single-engine),
                        # keeping the PV dependency chain inside the 2-step
                        # pipeline slack.
                        # ScalarE: true exp on the first XSPL columns
                        nc.scalar.activation(pT[:, 0:XSPL], ps_qk[:, 0:XSPL],
                                             Act.Exp)
                        # VectorE: fp8 exp2 bit trick on the rest --
                        # int8(round(A*s + B)) reinterpreted as fp8e4m3
                        nc.vector.tensor_scalar(
                            pT.bitcast(i8)[:, XSPL:JGRP * 512],
                            ps_qk[:, XSPL:JGRP * 512],
                            EXP_A8, EXP_B8, Alu.mult, Alu.add,
                        )
                    if s - 2 >= 0:  # PV for group s-2
                        ib, g = groups[s - 2]
                        if g == 0:
                            # [i, o|denom] accumulators, 4 i-subtiles, 1 bank
                            ps_pv_new = pv_ps_pool.tile(
                                [C, NSUB * (CO + 1)], f32, tag="pv")
                            pv_tiles[ib] = ps_pv_new
                        ps_pv = pv_tiles[ib]
                        pT = pT_tiles.pop(s - 2)
                        for idx in range(JGRP):
                            jt = g * JGRP + idx
                            for sub in range(NSUB):
                                # start/stop once per BANK: start=True clears
                                # the whole zero region; the other sub-slices
                                # rely on per-element has_written for
                                # overwrite-then-accumulate semantics.
                                nc.tensor.matmul(
                                    ps_pv[:, sub * (CO + 1):
                                          (sub + 1) * (CO + 1)],
                                    lhsT=pT[:, idx * 512 + sub * 128:
                                            idx * 512 + (sub + 1) * 128],
                                    rhs=vT3[:, jt, :],
                                    start=(jt == 0 and sub == 0),
                                    stop=(jt == NJT - 1 and sub == NSUB - 1),
                                )
                        if g == NGRP - 1:
                            emit_epilogue_a(ib)
                # last i-block's deferred phase B
                emit_epilogue_b(nib - 1)

    if split:
        _split_multiwaits(nc)
    return nc


def host_prep(inputs):
    """Fold scales/transposes on the host; returns the 8 per-core input maps."""
    x = np.ascontiguousarray(np.asarray(inputs["x"], dtype=np.float32))
    wq = np.asarray(inputs["wq"], dtype=np.float32)
    bq = np.asarray(inputs["bq"], dtype=np.float32)
    wk = np.asarray(inputs["wk"], dtype=np.float32)
    bk = np.asarray(inputs["bk"], dtype=np.float32)
    wv = np.asarray(inputs["wv"], dtype=np.float32)
    bv = np.asarray(inputs["bv"], dtype=np.float32)
    wo = np.asarray(inputs["wo"], dtype=np.float32)
    bo = np.asarray(inputs["bo"], dtype=np.float32)
    gamma = float(np.asarray(inputs["gamma"]).reshape(-1)[0])

    s = 1.0 / np.sqrt(np.float32(C))
    bf = ml_dtypes.bfloat16
    wqTs = wq.T * s                                                    # [128,64]
    wqT = np.concatenate([wqTs, wqTs], axis=1)                         # [128,128]
    wkT = np.concatenate([wk.T, wk.T], axis=1)                         # [128,128]
    wvT = wv.T                                                         # [128,64]
    woT_pad = np.zeros((C, C), np.float32)
    woT_pad[:CO, :] = gamma * wo.T                                     # rows 0:64
    wpack = np.concatenate([wqT, wkT, wvT, woT_pad], axis=1).astype(bf)
    bq_s = np.concatenate([bq * s, bq * s])
    bk_s = np.concatenate([bk, bk])
    gbo = gamma * (wo @ bv + bo)
    bpack = np.stack([bq_s, bk_s, gbo], axis=1).astype(np.float32)     # [128,3]

    xb = x.reshape(B, C, N)
    in_maps = []
    for b in range(B):
        in_maps.append({
            "x": np.ascontiguousarray(xb[b]),
            "xb": np.ascontiguousarray(xb[b].astype(bf)),
            "wpack": wpack, "bpack": bpack,
        })
    return in_maps


def run(inputs, trace=False, **kw):
    from concourse.bass_utils import run_bass_kernel_spmd

    if "nc" not in _CACHE:
        _CACHE["nc"] = build_nc()
    nc = _CACHE["nc"]
    in_maps = host_prep(inputs)
    try:
        res = run_bass_kernel_spmd(nc, in_maps, core_ids=list(range(NCORES)),
                                   trace=trace, **kw)
    except Exception:
        # transient device wedge (e.g. NRT_EXEC_UNIT_UNRECOVERABLE from an
        # earlier crashed process) -- retry once
        res = run_bass_kernel_spmd(nc, in_maps, core_ids=list(range(NCORES)),
                                   trace=trace, **kw)
    y = np.stack([np.asarray(res.results[b]["y"]) for b in range(B)])
    y = y.reshape(B, C, W, H).astype(np.float32)
    return y, res


def kernel(**inputs) -> np.ndarray:
    y, _ = run(inputs)
    return y



# revision 31
# speedup vs baseline: 1.0256x; 1.0256x over previous
"""ConvSelfAttention Trainium2 kernel.

Reference computation (per batch b, with x flattened to [C=128, N=4096]):
    q = wq @ x + bq        [64, N]   (scaled by 1/sqrt(128), folded into wq/bq)
    k = wk @ x + bk        [64, N]
    v = wv @ x + bv        [64, N]
    s[i,j] = sum_o q[o,i] k[o,j]
    p = softmax_j(s)
    out[o,i] = sum_j v[o,j] p[i,j]
    y = gamma * (wo @ out + bo) + x

Mapping (one batch per NeuronCore, 8 cores):
  - scores are built TRANSPOSED: sT[j,i] = sum_o k[o,j] q[o,i], j-tile (128) on
    partitions, i-block (512) on free dim; exp is SPLIT by score-group between
    ScalarE (Act.Exp, fp8e4m3 out) and VectorE (Schraudolph exp2 bit trick:
    int8(round(A*s + B)) bitcast to fp8e4m3) so the two engines share the
    16.8M-element softmax wall.  Emission is software-pipelined with a +2
    group skew (QK(s) | exp(s-1) | PV(s-2)) so the PE never head-of-line
    waits on exp and stays HAM-warm.
  - QK has K=64 (head dim), so q/k are kept DUPLICATED in both partition
    halves (duplication is free: the projection weight matrix is duplicated on
    the host) and consecutive j-tiles run CONCURRENTLY in the PE array via
    row tile_position (0,0)/(64,0).
  - PV uses the exp output pT (fp8) as the STATIONARY operand (M=128
    i-columns, FWL loads fp8 weights 4/cycle) streaming the ones-augmented
    fp8 V^T (N=65):
    psum[i, 0:64] accumulates attention output (transposed), psum[i, 64] the
    softmax denominator -- per-partition, so normalization is a plain
    tensor_scalar multiply and the reciprocal is a [128, 4] op.
  - the normalized [i, o] tile is PE-transposed back to [o, i] for the output
    projection; v's bias folds into the constant gamma*(wo@bv+bo) because
    sum_j p = 1; gamma folds into wo/bo on the host.

All matmuls run in bf16 (fp32 PSUM accumulation); exp runs on ScalarE in fp32.
"""

import sys

import numpy as np

try:
    import concourse  # noqa: F401
except ImportError:  # pragma: no cover
    sys.path.insert(0, "/opt/trn_rl_repo")

import ml_dtypes

B, C, CO, N = 8, 128, 64, 4096
W = H = 64
NCORES = 8
IBLK = 512          # query columns per i-block
NSUB = IBLK // 128  # 4 i-subtiles per i-block (PV stationary operand width)
NJT = N // 128      # 32 j-tiles of 128 keys
NIB = N // IBLK     # 8 i-blocks
JGRP = 2            # j-tiles per exp group (2 PSUM banks; 3 bufs in flight)
NGRP = NJT // JGRP  # 16 score groups per i-block
# fp8e4m3 exp2 bit trick: bits = round(8*log2e*s + (7*8 - C)), C tuned so the
# piecewise-linear 2^frac interpolation error is centered
EXP_A8 = 8.0 / float(np.log(2.0))
EXP_B8 = 7.0 * 8.0 - 0.35
# groups (of 16 per i-block) whose exp runs on VectorE; the rest on ScalarE.
# Boundary groups (14, 15, 0) stay on ScalarE so the i-block epilogue finds
# VectorE free.  Alternating 7/6 per i-block parity balances the two engines
# (VectorE also carries the epilogue normalize/copy work).
DVE_GROUPS_EVEN = frozenset({1, 3, 5, 7, 9, 11, 13})
DVE_GROUPS_ODD = frozenset({1, 3, 5, 7, 9, 11})

_CACHE = {}


def _split_multiwaits(nc):
    """Workaround for the pinned walrus: it accepts at most ONE semaphore wait
    per instruction (setupSyncWait: "Too many sync wait commands").  Hoist all
    but the last wait of any instruction onto single-wait NoOps inserted just
    before it in the same engine's stream — semantically identical (the engine
    blocks on each wait in turn before issuing the instruction)."""
    from concourse import mybir

    nsplit = 0
    for fn in nc.m.functions:
        for bb in fn.blocks:
            out = []
            for inst in bb.instructions:
                si = inst.sync_info
                if si is not None and si.on_wait is not None and len(si.on_wait) > 1:
                    waits = list(si.on_wait)
                    for i, w in enumerate(waits[:-1]):
                        out.append(mybir.InstNoOp(
                            name=f"{inst.name}-sw{i}",
                            engine=inst.engine,
                            sync_info=mybir.SyncInfo(on_wait=[w], on_update=[]),
                            bass_nofuse=True,
                        ))
                        nsplit += 1
                    si.on_wait = [waits[-1]]
                    inst.sync_info = si
                out.append(inst)
            bb.instructions = out
    return nsplit


def build_nc(debug=False, nib=NIB, use_bacc=False, split=True):
    from concourse import bacc, mybir
    import concourse.bass as bass
    import concourse.tile as tile
    from concourse.masks import make_identity

    f32 = mybir.dt.float32
    bf16 = mybir.dt.bfloat16
    f8 = mybir.dt.float8e4
    i8 = mybir.dt.int8
    Alu = mybir.AluOpType
    Act = mybir.ActivationFunctionType

    if use_bacc:
        nc = bacc.Bacc(None, target_bir_lowering=False, debug=debug)
    else:
        nc = bass.Bass()

    x_d = nc.dram_tensor("x", [C, N], f32, kind="ExternalInput")
    xb_d = nc.dram_tensor("xb", [C, N], bf16, kind="ExternalInput")  # host cast
    # packed bf16 weights: [wqT(128) | wkT(128) | wvT(64) | woT(128, rows 0:64)]
    wpack_d = nc.dram_tensor("wpack", [C, 448], bf16, kind="ExternalInput")
    # packed f32 scalars: [bq | bk | gbo]
    bpack_d = nc.dram_tensor("bpack", [C, 3], f32, kind="ExternalInput")
    y_d = nc.dram_tensor("y", [C, N], f32, kind="ExternalOutput")

    with tile.TileContext(nc) as tc:
        with (
            tc.tile_pool(name="consts", bufs=1) as consts,
            tc.tile_pool(name="big", bufs=1) as big,
            tc.tile_pool(name="pt", bufs=6) as pt_pool,
            tc.tile_pool(name="epi", bufs=2) as epi,
        ):
            # ---- x chunk 0 DMA first (critical path), then packed weights ----
            x_sb = big.tile([C, N], f32)
            x_bf = big.tile([C, N], bf16)
            nc.sync.dma_start(x_bf[:, 0:512], xb_d[:, 0:512])

            wpack = consts.tile([C, 448], bf16)
            nc.gpsimd.dma_start(wpack, wpack_d[:, :])
            bpack = consts.tile([C, 3], f32)
            nc.gpsimd.dma_start(bpack, bpack_d[:, :])
            wqT = wpack[:, 0:128]
            wkT = wpack[:, 128:256]
            wvT = wpack[:, 256:320]
            woT = wpack[0:CO, 320:448]
            bq_s = bpack[:, 0:1]
            bk_s = bpack[:, 1:2]
            gbo = bpack[:, 2:3]
            ident = consts.tile([C, C], bf16)
            make_identity(nc, ident)

            # warm the exp table set (~2.7us ACT_TABLE_LOAD) during the ramp;
            # memset source so no DMA dependency
            warm = consts.tile([C, 1], f32)
            nc.vector.memset(warm, 0.0)
            nc.scalar.activation(warm, warm, Act.Exp)

            # HAM pre-warm: ~3.4us of dummy matmuls while the weight/x DMAs
            # are in flight trips the PE clock gate to 8/8 (2.4 GHz) before
            # the real projections start (cold MMs run at 1.2 GHz)
            warm_x = consts.tile([C, 512], bf16)
            nc.vector.memset(warm_x, 0.0)

            # ---- x load + cast + projections, pipelined in 512-col chunks ----
            q_sb = big.tile([C, N], bf16)
            k_sb = big.tile([C, N], bf16)
            vT = big.tile([C, NJT * (CO + 1)], f8)  # 32 x [128, 65] tiles
            vT3 = vT.rearrange("p (t e) -> p t e", e=CO + 1)
            nc.vector.memset(vT3[:, :, CO:CO + 1], 1.0)

            with tc.tile_pool(name="setup_ps", bufs=4, space="PSUM") as setup_ps:
                ps_warm = setup_ps.tile([C, 512], f32, tag="warm", bufs=1)
                for _ in range(8):
                    nc.tensor.matmul(ps_warm, lhsT=warm_x[:, 0:128],
                                     rhs=warm_x, start=True, stop=True)
                for t in range(N // 512):
                    sl = slice(t * 512, (t + 1) * 512)
                    if t > 0:  # chunk 0 DMA already issued above
                        nc.sync.dma_start(x_bf[:, sl], xb_d[:, sl])
                    # x f32 (residual add) loads independently, off the
                    # projection critical path (gpsimd queue keeps the sync
                    # queue free for the xb chunks the projections wait on)
                    nc.gpsimd.dma_start(x_sb[:, sl], x_d[:, sl])
                    ps_q = setup_ps.tile([C, 512], f32, tag="proj")
                    nc.tensor.matmul(ps_q, lhsT=wqT, rhs=x_bf[:, sl],
                                     start=True, stop=True)
                    nc.vector.tensor_scalar_add(q_sb[:, sl], ps_q, bq_s)
                    ps_k = setup_ps.tile([C, 512], f32, tag="proj")
                    nc.tensor.matmul(ps_k, lhsT=wkT, rhs=x_bf[:, sl],
                                     start=True, stop=True)
                    # k bias on ScalarE (idle until the first exp) so setup
                    # is not paced by DVE alone
                    nc.scalar.activation(k_sb[:, sl], ps_k, Act.Identity,
                                         bias=bk_s)
                    ps_v = setup_ps.tile([C, 256], f32, tag="vt", bufs=2)
                    for tt in range(4):
                        nt = t * 4 + tt
                        nc.tensor.matmul(
                            ps_v[:, tt * CO:(tt + 1) * CO],
                            lhsT=x_bf[:, nt * 128:(nt + 1) * 128],
                            rhs=wvT,
                            start=True, stop=True,
                        )
                    nc.vector.tensor_copy(
                        vT3[:, t * 4:(t + 1) * 4, 0:CO],
                        ps_v.rearrange("p (t e) -> p t e", e=CO),
                    )

            # ---- main loop: software-pipelined QK(s) | exp(s-1) | PV(s-2) ---
            # The +2 emission skew keeps the PE stream free of head-of-line
            # waits (PV(g) only issues after exp(g) had a full step to run),
            # which keeps the PE HAM-warm at 2.4 GHz.
            with (
                tc.tile_pool(name="qk_ps", bufs=3, space="PSUM") as qk_ps_pool,
                tc.tile_pool(name="pv_ps", bufs=1, space="PSUM") as pv_ps_pool,
                tc.tile_pool(name="oc_ps", bufs=1, space="PSUM") as oc_ps_pool,
            ):
                groups = [(ib, g) for ib in range(nib) for g in range(NGRP)]
                nsteps = len(groups)
                qk_tiles, pT_tiles, pv_tiles, onT_tiles = {}, {}, {}, {}

                def emit_epilogue_a(ib):
                    # normalize only: these are the ops that hold the PV
                    # accumulator bank, so they go to the front of the queue
                    ps_pv = pv_tiles.pop(ib)
                    rden = epi.tile([C, NSUB], f32, tag="rden")
                    pv3 = ps_pv.rearrange("p (s e) -> p s e", e=CO + 1)
                    nc.vector.reciprocal(rden, pv3[:, :, CO])
                    onT = epi.tile([C, NSUB * CO], bf16, tag="onT")
                    onT_tiles[ib] = onT
                    for s in range(NSUB):
                        nc.vector.tensor_scalar_mul(
                            onT[:, s * CO:(s + 1) * CO],
                            pv3[:, s, 0:CO],
                            rden[:, s:s + 1],
                        )

                def emit_epilogue_b(ib):
                    # transpose, output projection, residual, store -- deferred
                    # two steps so the next i-block's exp groups queue first
                    isl = slice(ib * IBLK, (ib + 1) * IBLK)
                    onT = onT_tiles.pop(ib)
                    out_sb = epi.tile([CO, IBLK], bf16, tag="out")
                    for s in range(NSUB):
                        # transpose via a REGULAR matmul against identity
                        # (onT_s.T @ I): ~85ns vs ~275ns for transpose-mode
                        ps_t = oc_ps_pool.tile([CO, 128], f32, tag="oc")
                        nc.tensor.matmul(ps_t, lhsT=onT[:, s * CO:(s + 1) * CO],
                                         rhs=ident, start=True, stop=True)
                        nc.vector.tensor_copy(out_sb[:, s * 128:(s + 1) * 128],
                                              ps_t)
                    ps_oc = oc_ps_pool.tile([C, IBLK], f32, tag="oc")
                    nc.tensor.matmul(ps_oc, lhsT=woT, rhs=out_sb,
                                     start=True, stop=True)
                    y2 = epi.tile([C, IBLK], f32, tag="y2")
                    nc.vector.scalar_tensor_tensor(
                        out=y2, in0=ps_oc, scalar=gbo, in1=x_sb[:, isl],
                        op0=Alu.add, op1=Alu.add,
                    )
                    nc.sync.dma_start(y_d[:, isl], y2)

                for s in range(nsteps + 2):
                    # deferred epilogue phase B (two steps after its phase A)
                    if s - 4 >= 0 and groups[s - 4][1] == NGRP - 1:
                        emit_epilogue_b(groups[s - 4][0])
                    if s < nsteps:  # QK for group s
                        ib, g = groups[s]
                        isl = slice(ib * IBLK, (ib + 1) * IBLK)
                        ps_qk = qk_ps_pool.tile([128, JGRP * 512], f32)
                        qk_tiles[s] = ps_qk
                        for idx in range(JGRP):
                            jt = g * JGRP + idx
                            half = jt % 2  # alternate row halves -> PE pairs
                            hsl = slice(half * CO, half * CO + CO)
                            nc.tensor.matmul(
                                ps_qk[:, idx * 512:(idx + 1) * 512],
                                lhsT=k_sb[hsl, jt * 128:(jt + 1) * 128],
                                rhs=q_sb[hsl, isl],
                                start=True, stop=True,
                            )
                    if 0 <= s - 1 < nsteps:  # exp for group s-1
                        ib, g = groups[s - 1]
                        ps_qk = qk_tiles.pop(s - 1)
                        pT = pt_pool.tile([128, JGRP * 512], f8)
                        pT_tiles[s - 1] = pT
                        dve_set = (DVE_GROUPS_EVEN if ib % 2 == 0
                                   else DVE_GROUPS_ODD)
                        if g in dve_set:
                            # VectorE: fp8 exp2 bit trick --
                            # int8(round(A*s + B)) reinterpreted as fp8e4m3
                            nc.vector.tensor_scalar(
                                pT.bitcast(i8)[:, :], ps_qk[:, :],
                                EXP_A8, EXP_B8, Alu.mult, Alu.add,
                            )
                        else:
                            nc.scalar.activation(pT, ps_qk, Act.Exp)
                    if s - 2 >= 0:  # PV for group s-2
                        ib, g = groups[s - 2]
                        if g == 0:
                            # [i, o|denom] accumulators, 4 i-subtiles, 1 bank
                            ps_pv_new = pv_ps_pool.tile(
                                [C, NSUB * (CO + 1)], f32, tag="pv")
                            pv_tiles[ib] = ps_pv_new
                        ps_pv = pv_tiles[ib]
                        pT = pT_tiles.pop(s - 2)
                        for idx in range(JGRP):
                            jt = g * JGRP + idx
                            for sub in range(NSUB):
                                # start/stop once per BANK: start=True clears
                                # the whole zero region; the other sub-slices
                                # rely on per-element has_written for
                                # overwrite-then-accumulate semantics.
                                nc.tensor.matmul(
                                    ps_pv[:, sub * (CO + 1):
                                          (sub + 1) * (CO + 1)],
                                    lhsT=pT[:, idx * 512 + sub * 128:
                                            idx * 512 + (sub + 1) * 128],
                                    rhs=vT3[:, jt, :],
                                    start=(jt == 0 and sub == 0),
                                    stop=(jt == NJT - 1 and sub == NSUB - 1),
                                )
                        if g == NGRP - 1:
                            emit_epilogue_a(ib)
                # last i-block's deferred phase B
                emit_epilogue_b(nib - 1)

    if split:
        _split_multiwaits(nc)
    return nc


def host_prep(inputs):
    """Fold scales/transposes on the host; returns the 8 per-core input maps."""
    x = np.ascontiguousarray(np.asarray(inputs["x"], dtype=np.float32))
    wq = np.asarray(inputs["wq"], dtype=np.float32)
    bq = np.asarray(inputs["bq"], dtype=np.float32)
    wk = np.asarray(inputs["wk"], dtype=np.float32)
    bk = np.asarray(inputs["bk"], dtype=np.float32)
    wv = np.asarray(inputs["wv"], dtype=np.float32)
    bv = np.asarray(inputs["bv"], dtype=np.float32)
    wo = np.asarray(inputs["wo"], dtype=np.float32)
    bo = np.asarray(inputs["bo"], dtype=np.float32)
    gamma = float(np.asarray(inputs["gamma"]).reshape(-1)[0])

    s = 1.0 / np.sqrt(np.float32(C))
    bf = ml_dtypes.bfloat16
    wqTs = wq.T * s                                                    # [128,64]
    wqT = np.concatenate([wqTs, wqTs], axis=1)                         # [128,128]
    wkT = np.concatenate([wk.T, wk.T], axis=1)                         # [128,128]
    wvT = wv.T                                                         # [128,64]
    woT_pad = np.zeros((C, C), np.float32)
    woT_pad[:CO, :] = gamma * wo.T                                     # rows 0:64
    wpack = np.concatenate([wqT, wkT, wvT, woT_pad], axis=1).astype(bf)
    bq_s = np.concatenate([bq * s, bq * s])
    bk_s = np.concatenate([bk, bk])
    gbo = gamma * (wo @ bv + bo)
    bpack = np.stack([bq_s, bk_s, gbo], axis=1).astype(np.float32)     # [128,3]

    xb = x.reshape(B, C, N)
    in_maps = []
    for b in range(B):
        in_maps.append({
            "x": np.ascontiguousarray(xb[b]),
            "xb": np.ascontiguousarray(xb[b].astype(bf)),
            "wpack": wpack, "bpack": bpack,
        })
    return in_maps


def run(inputs, trace=False, **kw):
    from concourse.bass_utils import run_bass_kernel_spmd

    if "nc" not in _CACHE:
        _CACHE["nc"] = build_nc()
    nc = _CACHE["nc"]
    in_maps = host_prep(inputs)
    try:
        res = run_bass_kernel_spmd(nc, in_maps, core_ids=list(range(NCORES)),
                                   trace=trace, **kw)
    except Exception:
        # transient device wedge (e.g. NRT_EXEC_UNIT_UNRECOVERABLE from an
        # earlier crashed process) -- retry once
        res = run_bass_kernel_spmd(nc, in_maps, core_ids=list(range(NCORES)),
                                   trace=trace, **kw)
    y = np.stack([np.asarray(res.results[b]["y"]) for b in range(B)])
    y = y.reshape(B, C, W, H).astype(np.float32)
    return y, res


def kernel(**inputs) -> np.ndarray:
    y, _ = run(inputs)
    return y



# revision 36
# speedup vs baseline: 1.0917x; 1.0645x over previous
"""ConvSelfAttention Trainium2 kernel.

Reference computation (per batch b, with x flattened to [C=128, N=4096]):
    q = wq @ x + bq        [64, N]   (scaled by 1/sqrt(128), folded into wq/bq)
    k = wk @ x + bk        [64, N]
    v = wv @ x + bv        [64, N]
    s[i,j] = sum_o q[o,i] k[o,j]
    p = softmax_j(s)
    out[o,i] = sum_j v[o,j] p[i,j]
    y = gamma * (wo @ out + bo) + x

Mapping (one batch per NeuronCore, 8 cores):
  - scores are built TRANSPOSED: sT[j,i] = sum_o k[o,j] q[o,i], j-tile (128) on
    partitions, i-block (512) on free dim; exp is SPLIT by score-group between
    ScalarE (Act.Exp, fp8e4m3 out) and VectorE (Schraudolph exp2 bit trick:
    int8(round(A*s + B)) bitcast to fp8e4m3) so the two engines share the
    16.8M-element softmax wall.  Emission is software-pipelined with a +2
    group skew (QK(s) | exp(s-1) | PV(s-2)) so the PE never head-of-line
    waits on exp and stays HAM-warm.
  - QK has K=64 (head dim), so q/k are kept DUPLICATED in both partition
    halves (duplication is free: the projection weight matrix is duplicated on
    the host) and consecutive j-tiles run CONCURRENTLY in the PE array via
    row tile_position (0,0)/(64,0).
  - PV uses the exp output pT (fp8) as the STATIONARY operand (M=128
    i-columns, FWL loads fp8 weights 4/cycle) streaming the ones-augmented
    fp8 V^T (N=65):
    psum[i, 0:64] accumulates attention output (transposed), psum[i, 64] the
    softmax denominator -- per-partition, so normalization is a plain
    tensor_scalar multiply and the reciprocal is a [128, 4] op.
  - the normalized [i, o] tile is PE-transposed back to [o, i] for the output
    projection; v's bias folds into the constant gamma*(wo@bv+bo) because
    sum_j p = 1; gamma folds into wo/bo on the host.

QK/projections run in bf16, PV in fp8e4m3 (fp32 PSUM accumulation everywhere).
A dummy-matmul burst at kernel start pre-warms the PE HAM clock gate, and the
epilogue transpose uses a regular identity matmul instead of transpose-mode.
"""

import sys

import numpy as np

try:
    import concourse  # noqa: F401
except ImportError:  # pragma: no cover
    sys.path.insert(0, "/opt/trn_rl_repo")

import ml_dtypes

B, C, CO, N = 8, 128, 64, 4096
W = H = 64
NCORES = 8
IBLK = 512          # query columns per i-block
NSUB = IBLK // 128  # 4 i-subtiles per i-block (PV stationary operand width)
NJT = N // 128      # 32 j-tiles of 128 keys
NIB = N // IBLK     # 8 i-blocks
JGRP = 2            # j-tiles per exp group (2 PSUM banks; 3 bufs in flight)
NGRP = NJT // JGRP  # 16 score groups per i-block
# fp8e4m3 exp2 bit trick: bits = round(8*log2e*s + (7*8 - C)), C tuned so the
# piecewise-linear 2^frac interpolation error is centered
EXP_A8 = 8.0 / float(np.log(2.0))
EXP_B8 = 7.0 * 8.0 - 0.35
# groups (of 16 per i-block) whose exp runs on VectorE; the rest on ScalarE.
# Groups adjacent to the i-block boundary (14, 15, 0, 1, 3) stay on ScalarE
# so the epilogue's VectorE ops (normalize/copies, emitted between exp(1) and
# exp(3)) never delay an exp the PE is waiting on.  Alternating 7/6 per
# i-block parity balances the two engines.
DVE_GROUPS_EVEN = frozenset({2, 4, 6, 8, 10, 12, 13})
DVE_GROUPS_ODD = frozenset({2, 4, 6, 8, 10, 12})

_CACHE = {}


def _split_multiwaits(nc):
    """Workaround for the pinned walrus: it accepts at most ONE semaphore wait
    per instruction (setupSyncWait: "Too many sync wait commands").  Hoist all
    but the last wait of any instruction onto single-wait NoOps inserted just
    before it in the same engine's stream — semantically identical (the engine
    blocks on each wait in turn before issuing the instruction)."""
    from concourse import mybir

    nsplit = 0
    for fn in nc.m.functions:
        for bb in fn.blocks:
            out = []
            for inst in bb.instructions:
                si = inst.sync_info
                if si is not None and si.on_wait is not None and len(si.on_wait) > 1:
                    waits = list(si.on_wait)
                    for i, w in enumerate(waits[:-1]):
                        out.append(mybir.InstNoOp(
                            name=f"{inst.name}-sw{i}",
                            engine=inst.engine,
                            sync_info=mybir.SyncInfo(on_wait=[w], on_update=[]),
                            bass_nofuse=True,
                        ))
                        nsplit += 1
                    si.on_wait = [waits[-1]]
                    inst.sync_info = si
                out.append(inst)
            bb.instructions = out
    return nsplit


def build_nc(debug=False, nib=NIB, use_bacc=False, split=True):
    from concourse import bacc, mybir
    import concourse.bass as bass
    import concourse.tile as tile
    from concourse.masks import make_identity

    f32 = mybir.dt.float32
    bf16 = mybir.dt.bfloat16
    f8 = mybir.dt.float8e4
    i8 = mybir.dt.int8
    Alu = mybir.AluOpType
    Act = mybir.ActivationFunctionType

    if use_bacc:
        nc = bacc.Bacc(None, target_bir_lowering=False, debug=debug)
    else:
        nc = bass.Bass()

    x_d = nc.dram_tensor("x", [C, N], f32, kind="ExternalInput")
    xb_d = nc.dram_tensor("xb", [C, N], bf16, kind="ExternalInput")  # host cast
    # packed bf16 weights: [wqT(128) | wkT(128) | wvT(64) | woT(128, rows 0:64)]
    wpack_d = nc.dram_tensor("wpack", [C, 448], bf16, kind="ExternalInput")
    # packed f32 scalars: [bq | bk | gbo]
    bpack_d = nc.dram_tensor("bpack", [C, 3], f32, kind="ExternalInput")
    y_d = nc.dram_tensor("y", [C, N], f32, kind="ExternalOutput")

    with tile.TileContext(nc) as tc:
        with (
            tc.tile_pool(name="consts", bufs=1) as consts,
            tc.tile_pool(name="big", bufs=1) as big,
            tc.tile_pool(name="pt", bufs=6) as pt_pool,
            tc.tile_pool(name="epi", bufs=2) as epi,
        ):
            # ---- x chunk 0 DMA first (critical path), then packed weights ----
            x_sb = big.tile([C, N], f32)
            x_bf = big.tile([C, N], bf16)
            nc.sync.dma_start(x_bf[:, 0:512], xb_d[:, 0:512])

            wpack = consts.tile([C, 448], bf16)
            nc.gpsimd.dma_start(wpack, wpack_d[:, :])
            bpack = consts.tile([C, 3], f32)
            nc.gpsimd.dma_start(bpack, bpack_d[:, :])
            wqT = wpack[:, 0:128]
            wkT = wpack[:, 128:256]
            wvT = wpack[:, 256:320]
            woT = wpack[0:CO, 320:448]
            bq_s = bpack[:, 0:1]
            bk_s = bpack[:, 1:2]
            gbo = bpack[:, 2:3]
            ident = consts.tile([C, C], bf16)
            make_identity(nc, ident)

            # warm the exp table set (~2.7us ACT_TABLE_LOAD) during the ramp;
            # memset source so no DMA dependency
            warm = consts.tile([C, 1], f32)
            nc.vector.memset(warm, 0.0)
            nc.scalar.activation(warm, warm, Act.Exp)

            # HAM pre-warm: ~3.4us of dummy matmuls while the weight/x DMAs
            # are in flight trips the PE clock gate to 8/8 (2.4 GHz) before
            # the real projections start (cold MMs run at 1.2 GHz)
            warm_x = consts.tile([C, 512], bf16)
            nc.vector.memset(warm_x, 0.0)

            # ---- x load + cast + projections, pipelined in 512-col chunks ----
            q_sb = big.tile([C, N], bf16)
            k_sb = big.tile([C, N], bf16)
            vT = big.tile([C, NJT * (CO + 1)], f8)  # 32 x [128, 65] tiles
            vT3 = vT.rearrange("p (t e) -> p t e", e=CO + 1)
            nc.vector.memset(vT3[:, :, CO:CO + 1], 1.0)

            with tc.tile_pool(name="setup_ps", bufs=4, space="PSUM") as setup_ps:
                ps_warm = setup_ps.tile([C, 512], f32, tag="warm", bufs=1)
                for _ in range(8):
                    nc.tensor.matmul(ps_warm, lhsT=warm_x[:, 0:128],
                                     rhs=warm_x, start=True, stop=True)
                for t in range(N // 512):
                    sl = slice(t * 512, (t + 1) * 512)
                    if t > 0:  # chunk 0 DMA already issued above
                        nc.sync.dma_start(x_bf[:, sl], xb_d[:, sl])
                    # x f32 (residual add) loads independently, off the
                    # projection critical path (gpsimd queue keeps the sync
                    # queue free for the xb chunks the projections wait on)
                    nc.gpsimd.dma_start(x_sb[:, sl], x_d[:, sl])
                    ps_q = setup_ps.tile([C, 512], f32, tag="proj")
                    nc.tensor.matmul(ps_q, lhsT=wqT, rhs=x_bf[:, sl],
                                     start=True, stop=True)
                    nc.vector.tensor_scalar_add(q_sb[:, sl], ps_q, bq_s)
                    ps_k = setup_ps.tile([C, 512], f32, tag="proj")
                    nc.tensor.matmul(ps_k, lhsT=wkT, rhs=x_bf[:, sl],
                                     start=True, stop=True)
                    # k bias on ScalarE (idle until the first exp) so setup
                    # is not paced by DVE alone
                    nc.scalar.activation(k_sb[:, sl], ps_k, Act.Identity,
                                         bias=bk_s)
                    ps_v = setup_ps.tile([C, 256], f32, tag="vt", bufs=2)
                    for tt in range(4):
                        nt = t * 4 + tt
                        nc.tensor.matmul(
                            ps_v[:, tt * CO:(tt + 1) * CO],
                            lhsT=x_bf[:, nt * 128:(nt + 1) * 128],
                            rhs=wvT,
                            start=True, stop=True,
                        )
                    nc.vector.tensor_copy(
                        vT3[:, t * 4:(t + 1) * 4, 0:CO],
                        ps_v.rearrange("p (t e) -> p t e", e=CO),
                    )

            # ---- main loop: software-pipelined QK(s) | exp(s-1) | PV(s-2) ---
            # The +2 emission skew keeps the PE stream free of head-of-line
            # waits (PV(g) only issues after exp(g) had a full step to run),
            # which keeps the PE HAM-warm at 2.4 GHz.
            with (
                tc.tile_pool(name="qk_ps", bufs=3, space="PSUM") as qk_ps_pool,
                tc.tile_pool(name="pv_ps", bufs=1, space="PSUM") as pv_ps_pool,
                tc.tile_pool(name="oc_ps", bufs=1, space="PSUM") as oc_ps_pool,
            ):
                groups = [(ib, g) for ib in range(nib) for g in range(NGRP)]
                nsteps = len(groups)
                qk_tiles, pT_tiles, pv_tiles, onT_tiles = {}, {}, {}, {}

                def emit_epilogue_a(ib):
                    # normalize only: these are the ops that hold the PV
                    # accumulator bank, so they go to the front of the queue
                    ps_pv = pv_tiles.pop(ib)
                    rden = epi.tile([C, NSUB], f32, tag="rden")
                    pv3 = ps_pv.rearrange("p (s e) -> p s e", e=CO + 1)
                    nc.vector.reciprocal(rden, pv3[:, :, CO])
                    onT = epi.tile([C, NSUB * CO], bf16, tag="onT")
                    onT_tiles[ib] = onT
                    for s in range(NSUB):
                        nc.vector.tensor_scalar_mul(
                            onT[:, s * CO:(s + 1) * CO],
                            pv3[:, s, 0:CO],
                            rden[:, s:s + 1],
                        )

                def emit_epilogue_b(ib):
                    # transpose, output projection, residual, store -- deferred
                    # two steps so the next i-block's exp groups queue first
                    isl = slice(ib * IBLK, (ib + 1) * IBLK)
                    onT = onT_tiles.pop(ib)
                    out_sb = epi.tile([CO, IBLK], bf16, tag="out")
                    for s in range(NSUB):
                        # transpose via a REGULAR matmul against identity
                        # (onT_s.T @ I): ~85ns vs ~275ns for transpose-mode
                        ps_t = oc_ps_pool.tile([CO, 128], f32, tag="oc")
                        nc.tensor.matmul(ps_t, lhsT=onT[:, s * CO:(s + 1) * CO],
                                         rhs=ident, start=True, stop=True)
                        # copies split 2/2 across ScalarE/VectorE for balance
                        if s < 2:
                            nc.scalar.copy(out_sb[:, s * 128:(s + 1) * 128],
                                           ps_t)
                        else:
                            nc.vector.tensor_copy(
                                out_sb[:, s * 128:(s + 1) * 128], ps_t)
                    ps_oc = oc_ps_pool.tile([C, IBLK], f32, tag="oc")
                    nc.tensor.matmul(ps_oc, lhsT=woT, rhs=out_sb,
                                     start=True, stop=True)
                    y2 = epi.tile([C, IBLK], f32, tag="y2")
                    nc.vector.scalar_tensor_tensor(
                        out=y2, in0=ps_oc, scalar=gbo, in1=x_sb[:, isl],
                        op0=Alu.add, op1=Alu.add,
                    )
                    nc.sync.dma_start(y_d[:, isl], y2)

                for s in range(nsteps + 2):
                    # deferred epilogue phase B (two steps after its phase A)
                    if s - 4 >= 0 and groups[s - 4][1] == NGRP - 1:
                        emit_epilogue_b(groups[s - 4][0])
                    if s < nsteps:  # QK for group s
                        ib, g = groups[s]
                        isl = slice(ib * IBLK, (ib + 1) * IBLK)
                        ps_qk = qk_ps_pool.tile([128, JGRP * 512], f32)
                        qk_tiles[s] = ps_qk
                        for idx in range(JGRP):
                            jt = g * JGRP + idx
                            half = jt % 2  # alternate row halves -> PE pairs
                            hsl = slice(half * CO, half * CO + CO)
                            nc.tensor.matmul(
                                ps_qk[:, idx * 512:(idx + 1) * 512],
                                lhsT=k_sb[hsl, jt * 128:(jt + 1) * 128],
                                rhs=q_sb[hsl, isl],
                                start=True, stop=True,
                            )
                    if s < nsteps:  # exp for group s, right behind its QK:
                        # the engine picks it up the moment the scores land,
                        # widening PV(s)'s two-step slack
                        ib, g = groups[s]
                        ps_qk = qk_tiles.pop(s)
                        pT = pt_pool.tile([128, JGRP * 512], f8)
                        pT_tiles[s] = pT
                        dve_set = (DVE_GROUPS_EVEN if ib % 2 == 0
                                   else DVE_GROUPS_ODD)
                        if g in dve_set:
                            # VectorE: fp8 exp2 bit trick --
                            # int8(round(A*s + B)) reinterpreted as fp8e4m3
                            nc.vector.tensor_scalar(
                                pT.bitcast(i8)[:, :], ps_qk[:, :],
                                EXP_A8, EXP_B8, Alu.mult, Alu.add,
                            )
                        else:
                            nc.scalar.activation(pT, ps_qk, Act.Exp)
                    if s - 2 >= 0:  # PV for group s-2
                        ib, g = groups[s - 2]
                        if g == 0:
                            # [i, o|denom] accumulators, 4 i-subtiles, 1 bank
                            ps_pv_new = pv_ps_pool.tile(
                                [C, NSUB * (CO + 1)], f32, tag="pv")
                            pv_tiles[ib] = ps_pv_new
                        ps_pv = pv_tiles[ib]
                        pT = pT_tiles.pop(s - 2)
                        for idx in range(JGRP):
                            jt = g * JGRP + idx
                            for sub in range(NSUB):
                                # start/stop once per BANK: start=True clears
                                # the whole zero region; the other sub-slices
                                # rely on per-element has_written for
                                # overwrite-then-accumulate semantics.
                                nc.tensor.matmul(
                                    ps_pv[:, sub * (CO + 1):
                                          (sub + 1) * (CO + 1)],
                                    lhsT=pT[:, idx * 512 + sub * 128:
                                            idx * 512 + (sub + 1) * 128],
                                    rhs=vT3[:, jt, :],
                                    start=(jt == 0 and sub == 0),
                                    stop=(jt == NJT - 1 and sub == NSUB - 1),
                                )
                        if g == NGRP - 1:
                            emit_epilogue_a(ib)
                # last i-block's deferred phase B
                emit_epilogue_b(nib - 1)

    if split:
        _split_multiwaits(nc)
    return nc


def host_prep(inputs):
    """Fold scales/transposes on the host; returns the 8 per-core input maps."""
    x = np.ascontiguousarray(np.asarray(inputs["x"], dtype=np.float32))
    wq = np.asarray(inputs["wq"], dtype=np.float32)
    bq = np.asarray(inputs["bq"], dtype=np.float32)
    wk = np.asarray(inputs["wk"], dtype=np.float32)
    bk = np.asarray(inputs["bk"], dtype=np.float32)
    wv = np.asarray(inputs["wv"], dtype=np.float32)
    bv = np.asarray(inputs["bv"], dtype=np.float32)
    wo = np.asarray(inputs["wo"], dtype=np.float32)
    bo = np.asarray(inputs["bo"], dtype=np.float32)
    gamma = float(np.asarray(inputs["gamma"]).reshape(-1)[0])

    s = 1.0 / np.sqrt(np.float32(C))
    bf = ml_dtypes.bfloat16
    wqTs = wq.T * s                                                    # [128,64]
    wqT = np.concatenate([wqTs, wqTs], axis=1)                         # [128,128]
    wkT = np.concatenate([wk.T, wk.T], axis=1)                         # [128,128]
    wvT = wv.T                                                         # [128,64]
    woT_pad = np.zeros((C, C), np.float32)
    woT_pad[:CO, :] = gamma * wo.T                                     # rows 0:64
    wpack = np.concatenate([wqT, wkT, wvT, woT_pad], axis=1).astype(bf)
    bq_s = np.concatenate([bq * s, bq * s])
    bk_s = np.concatenate([bk, bk])
    gbo = gamma * (wo @ bv + bo)
    bpack = np.stack([bq_s, bk_s, gbo], axis=1).astype(np.float32)     # [128,3]

    xb = x.reshape(B, C, N)
    in_maps = []
    for b in range(B):
        in_maps.append({
            "x": np.ascontiguousarray(xb[b]),
            "xb": np.ascontiguousarray(xb[b].astype(bf)),
            "wpack": wpack, "bpack": bpack,
        })
    return in_maps


def run(inputs, trace=False, **kw):
    from concourse.bass_utils import run_bass_kernel_spmd

    if "nc" not in _CACHE:
        _CACHE["nc"] = build_nc()
    nc = _CACHE["nc"]
    in_maps = host_prep(inputs)
    try:
        res = run_bass_kernel_spmd(nc, in_maps, core_ids=list(range(NCORES)),
                                   trace=trace, **kw)
    except Exception:
        # transient device wedge (e.g. NRT_EXEC_UNIT_UNRECOVERABLE from an
        # earlier crashed process) -- retry once
        res = run_bass_kernel_spmd(nc, in_maps, core_ids=list(range(NCORES)),
                                   trace=trace, **kw)
    y = np.stack([np.asarray(res.results[b]["y"]) for b in range(B)])
    y = y.reshape(B, C, W, H).astype(np.float32)
    return y, res


def kernel(**inputs) -> np.ndarray:
    y, _ = run(inputs)
    return y

